# revision 4
# baseline (speedup 1.0000x reference)
"""GCN layer kernel for 8 trn2 NeuronCores (SPMD, single launch).

Math:  out = D^-1/2 (A+I) D^-1/2 X W^T + b
Identity: the dense layer commutes with the diagonal scalings:
    out = D^-1/2 (A+I) D^-1/2 (X W^T) + b
so U = X@W^T (tiny) is computed first, then one big matmul A_hat @ (dinv*U).

Distribution: row-shard A_hat = A+I across 8 cores (strip = 1024 rows).
The host supplies each core's strip TRANSPOSED, CENTERED (-0.5), cast to
fp8e4m3, and packed per k-tile in a PER-CORE-ROTATED k order (local
k-tiles first: tile q <-> global tile (c*8+q)%64), which
  * quarters HBM traffic vs fp32 (8.4MB/core, ~24us at 358GB/s roofline),
  * makes every stream DMA a 512KB contiguous-per-partition transfer,
  * puts the contraction dim k on partitions (no on-device transposes),
  * lets phase 2 start on the 8 local k-tiles BEFORE the dinv AllGather
    lands (local dinv is known without any communication).
Centering halves the fp8 quantization error; the rank-1 0.5*ones term is
restored exactly via a colsum(Y) correction, and degrees via a +N/2 shift.

Per core:
  phase 1 (overlapped): stream the fp8 strip in 16 512KB DMAs on the two
      HWDGE queues; degrees deg[i] = sum_k at[k, i] via DoubleRow fp8
      ones-matmuls (2 k-tiles per matmul, PSUM accum); local
      dinv = (deg+4096)^-1/2 computed BEFORE the collective so the gather
      ships final dinv values.
  AllGather (the only collective): 1024 local dinv -> full 8192.
  across the collective window: U = X@W^T (X^T loads after A, off the
      critical HBM window), local-k phase-2 matmuls, and filler matmuls
      keep the PE clock-gate warm.
  phase 2: Y = dinv*U (bf16, in place, per k-tile, pipelined);
      Z^T[f, i] = sum_k Y[k, f] at[k, i] (Y stationary, fp8 moving), half
      0 finishing first so its epilogue overlaps half 1's matmuls;
      colsum(Y) accumulated on GpSimd for the centering correction;
      epilogue: PE-transpose Z^T tiles, out = dinvL*Z + dinvL*0.5*s + b.

A is read from HBM exactly once, in fp8.
"""

import numpy as np
import ml_dtypes

N = 8192          # nodes
F = 128           # in/out feature dim
NCORES = 8
SR = N // NCORES  # strip rows per core = 1024
P = 128           # partitions / tile edge
IT = SR // P      # 8 local row tiles
JT = N // P       # 64 contraction tiles
HC = 512          # phase-2 / degree stream chunk (one PSUM bank of fp32)
NCHUNK = 16       # A-stream DMA chunks (512KB each)
KPC = JT // NCHUNK  # k-tiles per stream chunk = 4
MEAN = 0.5        # subtracted from A_hat on host, restored on device
NDUM = 24         # PE warm-keeper matmuls across the collective window

_CACHE = {}


def _build_nc():
    import concourse.mybir as mybir
    from concourse import bass
    from concourse.tile import TileContext

    f32 = mybir.dt.float32
    bf16 = mybir.dt.bfloat16
    f8 = mybir.dt.float8e4
    AF = mybir.ActivationFunctionType
    DR = mybir.MatmulPerfMode.DoubleRow

    nc = bass.Bass(num_devices=NCORES)

    At_d = nc.declare_dram_parameter("at_f8", [P, N * SR // P], f8, False)
    Xt_d = nc.declare_dram_parameter("xt_bf", [P, N], bf16, False)    # X^T
    Cpk_d = nc.declare_dram_parameter("cpack", [P, 440], f32, False)
    Wpk_d = nc.declare_dram_parameter("wpack", [P, 192], bf16, False)
    out = nc.declare_dram_parameter("out", [SR, F], f32, True)

    degL = nc.dram_tensor("deg_local", [IT, P], f32)
    degA = nc.dram_tensor("deg_all", [JT, P], f32, addr_space="Shared")

    rg = [list(range(NCORES))]

    with TileContext(nc) as tc:
        with tc.tile_pool(name="const", bufs=1) as constp, \
             tc.tile_pool(name="big", bufs=1) as bigp, \
             tc.tile_pool(name="small", bufs=1) as smallp, \
             tc.tile_pool(name="outs", bufs=3) as outp, \
             tc.tile_pool(name="pdeg", bufs=1, space="PSUM") as pdeg, \
             tc.tile_pool(name="pu", bufs=2, space="PSUM") as pu, \
             tc.tile_pool(name="pzt", bufs=2, space="PSUM") as pzt, \
             tc.tile_pool(name="ptr", bufs=2, space="PSUM") as ptr:

            # ---- packed constants (2 small DMAs ahead of the A stream) --
            cpk = constp.tile([P, 440], f32)
            nc.sync.dma_start(out=cpk[:, :], in_=Cpk_d[:, :])
            wpk = constp.tile([P, 192], bf16)
            nc.sync.dma_start(out=wpk[:, :], in_=Wpk_d[:, :])
            ident = cpk[:, 0:P]              # I_128 (fp32)
            bb_sb = cpk[:, P:2 * P]          # bias broadcast [128, F]
            perm56 = cpk[0:JT, 256:312]      # rot-order one-hot [64, 56]
            onesF = cpk[:, 312:440]          # all-ones fp32 block
            wt_sb = wpk[:, 0:F]              # W^T bf16 [128, 128]
            ones3 = constp.tile([P, 2, 64], f8)
            nc.vector.memset(ones3[:, :, :], 1.0)

            # ---- persistent big buffers ----
            At = bigp.tile([P, JT, SR], f8)        # packed strip, fp8
            Usb = bigp.tile([P, N], bf16)          # U tiles, then Y
            xt_sb = bigp.tile([P, N], bf16)

            # ---- stream A strip: 16 x 512KB on the two HWDGE queues ----
            for ch in range(NCHUNK):
                eng = nc.sync if ch % 2 == 0 else nc.scalar
                eng.dma_start(
                    out=At[:, ch * KPC:(ch + 1) * KPC, :],
                    in_=At_d[:, ch * KPC * SR:(ch + 1) * KPC * SR],
                )
            # X^T after A on the sync queue (4 chunks so U can start early)
            for xc in range(4):
                nc.sync.dma_start(
                    out=xt_sb[:, xc * (N // 4):(xc + 1) * (N // 4)],
                    in_=Xt_d[:, xc * (N // 4):(xc + 1) * (N // 4)],
                )

            # ---- degrees: deg[i] = sum_k at[k, i]; DoubleRow fp8 matmuls
            # contract two k-tiles (256 deep) per instruction ----
            degPs = [pdeg.tile([64, HC], f32, name=f"degP{h}", bufs=1)
                     for h in range(2)]
            for j in range(JT // 2):
                for h in range(2):
                    nc.tensor.matmul(
                        degPs[h][:, :],
                        ones3[:, :, :],
                        At[:, 2 * j:2 * j + 2, h * HC:(h + 1) * HC],
                        start=(j == 0), stop=(j == JT // 2 - 1),
                        perf_mode=DR,
                    )

            # ---- local dinv before the gather ----
            degS = smallp.tile([1, SR], f32)
            nc.scalar.copy(degS[0:1, 0:HC], degPs[0][0:1, :])
            nc.scalar.copy(degS[0:1, HC:SR], degPs[1][0:1, :])
            nc.vector.tensor_scalar_add(degS[:, :], degS[:, :],
                                        float(N * MEAN))
            sqS = smallp.tile([1, SR], f32)
            nc.scalar.activation(sqS[:, :], degS[:, :], AF.Sqrt)
            dinvS = smallp.tile([1, SR], f32)
            nc.vector.reciprocal(dinvS[:, :], sqS[:, :])
            nc.gpsimd.dma_start(out=degL[:, :], in_=dinvS[:, :])

            # ---- AllGather local dinv -> full dinv ----
            nc.gpsimd.collective_compute(
                "AllGather", mybir.AluOpType.bypass,
                replica_groups=rg,
                ins=[degL[:, :]], outs=[degA[:, :]],
            )

            # ---- local dinvT [128, 0:8] without the collective ----
            t8 = smallp.tile([IT, P], f32)
            nc.scalar.dma_start(out=t8[:, :], in_=dinvS[:, :])
            dinvT = smallp.tile([P, JT], f32)
            tpl = ptr.tile([P, IT], f32, tag="tr")
            nc.tensor.matmul(tpl[:, :], t8[:, :], ident[0:IT, 0:IT],
                             start=True, stop=True)
            nc.vector.tensor_copy(dinvT[:, 0:IT], tpl[:, :])

            # ---- U = X @ W^T across the collective window ----
            for jt in range(JT):
                up = pu.tile([P, F], f32)
                nc.tensor.matmul(
                    up[:, :], xt_sb[:, jt * P:(jt + 1) * P], wt_sb[:, :],
                    start=True, stop=True,
                )
                nc.vector.tensor_copy(Usb[:, jt * F:(jt + 1) * F], up[:, :])

            # ---- phase 2 setup ----
            zts = [pzt.tile([P, HC], f32, name=f"zt{h}", bufs=1)
                   for h in range(2)]
            acc = smallp.tile([P, F], f32)
            nc.gpsimd.memset(acc[:, :], 0.0)

            def scale_y(q):
                ut = Usb[:, q * F:(q + 1) * F]
                nc.vector.tensor_scalar_mul(ut, ut, dinvT[:, q:q + 1])
                nc.gpsimd.tensor_add(acc[:, :], acc[:, :], ut)
                return ut

            def mm(h, q, start, stop):
                nc.tensor.matmul(
                    zts[h][:, :], Usb[:, q * F:(q + 1) * F],
                    At[:, q:q + 1, h * HC:(h + 1) * HC],
                    start=start, stop=stop,
                )

            # local k-tiles (rot order puts them first): no gather needed
            for q in range(IT):
                scale_y(q)
                mm(0, q, q == 0, False)
                mm(1, q, q == 0, False)

            # warm keepers (into the drained degree banks)
            for d in range(NDUM):
                nc.tensor.matmul(degPs[d % 2][:, :], ones3[:, :, :],
                                 At[:, 0:2, 0:HC], start=True, stop=True,
                                 perf_mode=DR)

            # ---- post-gather: dinvT[:, 8:64] via permuting matmul ----
            dinvG = smallp.tile([JT, P], f32)
            nc.sync.dma_start(out=dinvG[:, :], in_=degA[:, :])
            tpg = ptr.tile([P, JT - IT], f32, tag="tr")
            nc.tensor.matmul(tpg[:, :], dinvG[:, :], perm56[:, :],
                             start=True, stop=True)
            nc.vector.tensor_copy(dinvT[:, IT:JT], tpg[:, :])

            # ---- pass B: half 0 over remote k-tiles (finishes first) ----
            for q in range(IT, JT):
                scale_y(q)
                mm(0, q, False, q == JT - 1)

            # ---- correction term: s = MEAN * colsum(Y), broadcast ----
            sps = ptr.tile([1, F], f32, tag="tr")
            nc.tensor.matmul(sps[0:1, :], onesF[:, 0:1], acc[:, :],
                             start=True, stop=True)
            s_sb = smallp.tile([1, F], f32)
            nc.scalar.mul(s_sb[:, :], sps[0:1, :], float(MEAN))
            sbps = ptr.tile([P, F], f32, tag="tr")
            nc.tensor.matmul(sbps[:, :], onesF[0:1, :], s_sb[:, :],
                             start=True, stop=True)
            s_bc = smallp.tile([P, F], f32)
            nc.vector.tensor_copy(s_bc[:, :], sbps[:, :])
            # bcb[it] = dinvL * s_bc + b   (dinvL = dinvT[:, 0:8])
            bcb = smallp.tile([P, IT * F], f32)
            for it in range(IT):
                nc.vector.tensor_scalar_mul(
                    bcb[:, it * F:(it + 1) * F], s_bc[:, :],
                    dinvT[:, it:it + 1])
                nc.vector.tensor_add(
                    bcb[:, it * F:(it + 1) * F],
                    bcb[:, it * F:(it + 1) * F], bb_sb[:, :])

            # ---- epilogue: transpose back, row scale, correction ----
            def epi(h):
                ztS = outp.tile([P, HC], f32)
                nc.vector.tensor_copy(ztS[:, :], zts[h][:, :])
                for q in range(4):
                    it = h * 4 + q
                    tp = ptr.tile([P, P], f32, tag="tr")
                    nc.tensor.transpose(tp[:, :], ztS[:, q * P:(q + 1) * P],
                                        ident[:, :])
                    o = outp.tile([P, F], f32)
                    nc.vector.tensor_scalar_mul(o[:, :], tp[:, :],
                                                dinvT[:, it:it + 1])
                    nc.vector.tensor_add(o[:, :], o[:, :],
                                         bcb[:, it * F:(it + 1) * F])
                    nc.scalar.dma_start(out=out[it * P:(it + 1) * P, :],
                                        in_=o[:, :])

            # epi(0) transposes slot in before pass C's matmuls, so half
            # 0's drain overlaps half 1's accumulation
            epi(0)

            # ---- pass C: half 1 over remote k-tiles ----
            for q in range(IT, JT):
                mm(1, q, False, q == JT - 1)
            epi(1)

    return nc


_NO_SPLIT_TYPES = ("InstEventSemaphore", "InstSemaphore", "InstTrigger")


def _split_drain_waits(nc, max_waits=1):
    """This walrus build only encodes one sem-wait per instruction; hoist
    extras onto preceding same-engine NOPs (monotonic sems => equivalent)."""
    import concourse.mybir as mybir
    for fn in nc.m.functions:
        for blk in fn.blocks:
            newlist = []
            for ins in blk.instructions:
                si = getattr(ins, "sync_info", None)
                tname = type(ins).__name__
                if si is not None and si.on_wait and len(si.on_wait) > max_waits \
                        and not any(tname.startswith(t) for t in _NO_SPLIT_TYPES):
                    waits = list(si.on_wait)
                    for j, w in enumerate(waits[max_waits:]):
                        newlist.append(mybir.InstNoOp(
                            name=f"{ins.name}-w{j}", engine=ins.engine,
                            ins=[], outs=[],
                            sync_info=mybir.SyncInfo(on_wait=[w], on_update=[]),
                        ))
                    si.on_wait = waits[:max_waits]
                newlist.append(ins)
            blk.instructions[:] = newlist


def _get_nc():
    if "nc" not in _CACHE:
        nc = _build_nc()
        _split_drain_waits(nc)
        _CACHE["nc"] = nc
    return _CACHE["nc"]


def _make_in_maps(X, A, W, b):
    bf16 = ml_dtypes.bfloat16
    f8 = ml_dtypes.float8_e4m3
    X = np.ascontiguousarray(np.asarray(X, dtype=np.float32))
    A = np.ascontiguousarray(np.asarray(A, dtype=np.float32))
    W = np.ascontiguousarray(np.asarray(W, dtype=np.float32))
    b = np.ascontiguousarray(np.asarray(b, dtype=np.float32))
    Xt = np.ascontiguousarray(X.T)  # [128, 8192] fp32

    cpack = np.zeros((P, 440), dtype=np.float32)
    cpack[:, 0:P] = np.eye(P, dtype=np.float32)
    cpack[:, P:2 * P] = np.tile(b[None, :], (P, 1))
    cpack[:, 312:440] = 1.0
    wpack = np.zeros((P, 192), dtype=np.float32)
    wpack[:, 0:F] = W.T
    wpack = wpack.astype(bf16)

    idx = np.arange(SR)
    in_maps = []
    for c in range(NCORES):
        gidx = (c * IT + np.arange(JT)) % JT     # rotated k-tile order
        at = A[c * SR:(c + 1) * SR, :].T.astype(np.float32)  # [N, SR]
        at[c * SR + idx, idx] += np.float32(1.0)             # self loops
        at -= np.float32(MEAN)                               # centering
        at8 = at.astype(f8)
        # pack rotated k-tiles: pk[p, q*SR + i] = at[gidx[q]*P + p, i]
        pk = np.ascontiguousarray(
            at8.reshape(JT, P, SR)[gidx].transpose(1, 0, 2)
            .reshape(P, N * SR // P))
        xt_rot = np.ascontiguousarray(
            Xt.reshape(P, JT, P)[:, gidx, :].reshape(P, N)).astype(bf16)
        cp = cpack.copy()
        cp[gidx[IT:], 256 + np.arange(JT - IT)] = 1.0  # perm56 one-hot
        in_maps.append({
            "at_f8": pk,
            "xt_bf": xt_rot,
            "cpack": cp,
            "wpack": wpack,
        })
    return in_maps


def _install_ntff_hook():
    """This image's antenv lacks axon_hooks; synthesize it so trace=True
    can reach the terminal's NTFF capture via the libaxon ctypes hook."""
    import sys
    import types
    if "antenv.axon_hooks" in sys.modules:
        return
    try:
        from trn_agent_boot.trn_boot import _ntff_profile_via_ctypes
        hook = _ntff_profile_via_ctypes("/opt/axon/libaxon_pjrt.so")
    except Exception:
        hook = None
    mod = types.ModuleType("antenv.axon_hooks")
    mod._hook = hook
    mod.get_axon_ntff_profile_hook = lambda: mod._hook
    def _set(h):
        mod._hook = h
    mod.set_axon_ntff_profile_hook = _set
    sys.modules["antenv.axon_hooks"] = mod
    import antenv
    antenv.axon_hooks = mod
    # the artifact upload needs a bucket this sandbox doesn't have
    import concourse.bass_utils as bu
    bu.upload_artifacts = lambda tmpdir: f"local:{tmpdir}"


def run(X, A, W, b, trace=False, **trace_kwargs):
    """Run on hardware; returns (output, BassKernelResults)."""
    from concourse.bass_utils import run_bass_kernel_spmd
    if trace:
        _install_ntff_hook()
    nc = _get_nc()
    in_maps = _make_in_maps(X, A, W, b)
    res = run_bass_kernel_spmd(nc, in_maps, list(range(NCORES)),
                               trace=trace, **trace_kwargs)
    outs = [np.asarray(res.results[c]["out"], dtype=np.float32)
            for c in range(NCORES)]
    return np.concatenate(outs, axis=0), res


def kernel(X, A, W, b):
    out, _ = run(X, A, W, b, trace=False)
    return out
